# revision 16
# baseline (speedup 1.0000x reference)
"""Trainium2 Bass kernel for the coverage-attention module.

Strategy (data-parallel over batch B=8, one batch per NeuronCore):
  - Host precompute (cheap, layout-only / tiny GEMMs):
      * query = hidden @ W_h + b_h                      [8, 512]
      * M     = einsum(conv_w, W_att) -> [121, 512]     (fuses the 11x11 conv
        with the channel-mixing einsum: coverage_alpha = im2col(alpha_sum) @ M)
      * X     = im2col of padded alpha_sum -> [8, 121, 16384]
  - Device (per core): conv-as-matmul + trans add + tanh + energy matvec on
    the tensor engine, then a local softmax (the global max subtraction is
    numerically equivalent to a per-sample max shift: softmax is shift
    invariant and the 1e-10 denominator guard perturbs at ~1e-13 relative,
    far below fp32 resolution).
  - context_vector = einsum(alpha * (alpha > 0.02), cnn_features).  Since
    sum(alpha) <= 1, at most 49 positions can exceed 0.02; for generic
    inputs none do and the result is exactly 0.  The mask is checked on the
    host from the returned alpha; only if any position passes is a second
    (dense matvec) kernel built and run.
"""

import numpy as np
from contextlib import ExitStack

import concourse.bass as bass
import concourse.tile as tile
from concourse import bacc, mybir, bass_isa
from concourse import bass_utils

B, C, H, W = 8, 684, 64, 256
HID, A, CONV_CH, KS = 256, 512, 512, 11
HW = H * W              # 16384
TAPS = KS * KS          # 121
NCH = A // 128          # 4 chunks of the attention dim
NT4 = 8                 # big DMA tiles along hw (2048 wide)
NTT = 4                 # compute subtiles per big tile (512 wide)
TBIG = HW // NT4        # 2048
TSUB = TBIG // NTT      # 512
F32 = mybir.dt.float32
BF16 = mybir.dt.bfloat16
AF = mybir.ActivationFunctionType
USE_PE_FRAC = True


def _build_main_nc():
    nc = bacc.Bacc("TRN2", target_bir_lowering=False, debug=False, num_devices=B)

    xt_d = nc.dram_tensor("xt", [TAPS, HW], BF16, kind="ExternalInput").ap()
    tr_d = nc.dram_tensor("tr", [A, HW], BF16, kind="ExternalInput").ap()
    qp_d = nc.dram_tensor("qp", [128, NCH], F32, kind="ExternalInput").ap()
    mm_d = nc.dram_tensor("mm", [TAPS, A], BF16, kind="ExternalInput").ap()
    wa_d = nc.dram_tensor("wa", [128, NCH * 32], BF16, kind="ExternalInput").ap()
    id_d = nc.dram_tensor("id128", [128, 128], BF16, kind="ExternalInput").ap()
    am_d = nc.dram_tensor("am", [HW], F32, kind="ExternalInput").ap()
    im_d = nc.dram_tensor("im", [HW], F32, kind="ExternalInput").ap()
    alpha_d = nc.dram_tensor("alpha_o", [HW], F32, kind="ExternalOutput").ap()
    nas_d = nc.dram_tensor("nas_o", [HW], F32, kind="ExternalOutput").ap()
    e_dram = nc.dram_tensor("e_scratch", [NCH, HW], F32, kind="Internal").ap()

    with tile.TileContext(nc) as tc, ExitStack() as ctx:
        consts = ctx.enter_context(tc.tile_pool(name="consts", bufs=1))
        xpool = ctx.enter_context(tc.tile_pool(name="xpool", bufs=3))
        trpool = ctx.enter_context(tc.tile_pool(name="trpool", bufs=12))
        prepool = ctx.enter_context(tc.tile_pool(name="prepool", bufs=3))
        scpool = ctx.enter_context(tc.tile_pool(name="scpool", bufs=6))
        evpool = ctx.enter_context(tc.tile_pool(name="evpool", bufs=2))
        smpool = ctx.enter_context(tc.tile_pool(name="smpool", bufs=1))
        cvps = ctx.enter_context(tc.tile_pool(name="cvps", bufs=3, space="PSUM"))
        epps = ctx.enter_context(tc.tile_pool(name="epps", bufs=2, space="PSUM"))

        mm_sb = consts.tile([TAPS, A], BF16)
        nc.sync.dma_start(out=mm_sb, in_=mm_d)
        qp_sb = consts.tile([128, NCH], F32)
        nc.sync.dma_start(out=qp_sb, in_=qp_d)
        wa_sb = consts.tile([128, NCH * 32], BF16)
        nc.sync.dma_start(out=wa_sb, in_=wa_d)
        id_sb = consts.tile([128, 128], BF16)
        nc.sync.dma_start(out=id_sb, in_=id_d)
        im128 = consts.tile([128, 128], F32)
        nc.sync.dma_start(out=im128, in_=im_d.rearrange("(p f) -> p f", p=128))
        am128 = consts.tile([128, 128], F32)
        nc.sync.dma_start(out=am128, in_=am_d.rearrange("(p f) -> p f", p=128))

        for t4 in range(NT4):
            xt_t = xpool.tile([TAPS, TBIG], BF16, tag="xt")
            nc.sync.dma_start(out=xt_t, in_=xt_d[:, t4 * TBIG:(t4 + 1) * TBIG])
            tr_ts = []
            for c in range(NCH):
                tr_t = trpool.tile([128, TBIG], BF16, tag="tr")
                nc.sync.dma_start(
                    out=tr_t,
                    in_=tr_d[c * 128:(c + 1) * 128, t4 * TBIG:(t4 + 1) * TBIG],
                )
                tr_ts.append(tr_t)
            # sc_ap(c, tt) -> AP of the [128, 512] tanh(score) slice
            sc_aps = {}
            for c in range(NCH):
                use_pe = USE_PE_FRAC and (t4 * NCH + c) % 2 == 0
                if use_pe:
                    # trans-add on the tensor engine (identity accumulate),
                    # tanh straight out of PSUM
                    for j in range(TBIG // 1024):
                        lo = j * 1024
                        cv = cvps.tile([128, 1024], F32, tag="cv")
                        for s in range(2):
                            nc.tensor.matmul(
                                cv[:, s * TSUB:(s + 1) * TSUB],
                                mm_sb[:, c * 128:(c + 1) * 128],
                                xt_t[:, lo + s * TSUB:lo + (s + 1) * TSUB],
                                start=True,
                                stop=True,
                            )
                        for s in range(2):
                            nc.tensor.matmul(
                                cv[:, s * TSUB:(s + 1) * TSUB],
                                id_sb,
                                tr_ts[c][:, lo + s * TSUB:lo + (s + 1) * TSUB],
                                start=False,
                                stop=True,
                                skip_group_check=True,
                            )
                        sc1k = scpool.tile([128, 1024], BF16, tag="sc1k")
                        nc.scalar.activation(
                            sc1k, cv, AF.Tanh, bias=qp_sb[:, c:c + 1], scale=1.0
                        )
                        sc_aps[(c, 2 * j)] = sc1k[:, 0:TSUB]
                        sc_aps[(c, 2 * j + 1)] = sc1k[:, TSUB:1024]
                else:
                    # trans-add on the vector engine, tanh from SBUF
                    pre2k = prepool.tile([128, TBIG], F32, tag="pre")
                    for j in range(TBIG // 1024):
                        lo = j * 1024
                        cv = cvps.tile([128, 1024], F32, tag="cv")
                        for s in range(2):
                            nc.tensor.matmul(
                                cv[:, s * TSUB:(s + 1) * TSUB],
                                mm_sb[:, c * 128:(c + 1) * 128],
                                xt_t[:, lo + s * TSUB:lo + (s + 1) * TSUB],
                                start=True,
                                stop=True,
                            )
                        nc.vector.tensor_add(
                            pre2k[:, lo:lo + 1024], cv, tr_ts[c][:, lo:lo + 1024]
                        )
                    sc2k = scpool.tile([128, TBIG], BF16, tag="sc")
                    nc.scalar.activation(
                        sc2k, pre2k, AF.Tanh, bias=qp_sb[:, c:c + 1], scale=1.0
                    )
                    for tt in range(NTT):
                        sc_aps[(c, tt)] = sc2k[:, tt * TSUB:(tt + 1) * TSUB]
            for tt in range(NTT):
                t = t4 * NTT + tt
                ep = epps.tile([128, TSUB], F32, tag="ep")
                for c in range(NCH):
                    nc.tensor.matmul(
                        ep[32 * c:32 * (c + 1), :],
                        wa_sb[:, c * 32:(c + 1) * 32],
                        sc_aps[(c, tt)],
                        start=True,
                        stop=True,
                        tile_position=(0, 32 * c),
                    )
                ev = evpool.tile([97, TSUB], F32, tag="ev")
                nc.vector.tensor_copy(ev, ep[0:97, :])
                nc.sync.dma_start(out=e_dram[:, t * TSUB:(t + 1) * TSUB],
                                  in_=ev[::32, :])

        # --- softmax over the 16384 positions, done on a [128, 128] layout ---
        ch_ts = []
        for c in range(NCH):
            ch_t = smpool.tile([128, 128], F32, name=f"ch{c}", tag=f"ch{c}")
            nc.sync.dma_start(out=ch_t,
                              in_=e_dram[c].rearrange("(p f) -> p f", p=128))
            ch_ts.append(ch_t)
        e01 = smpool.tile([128, 128], F32)
        nc.vector.tensor_add(e01, ch_ts[0], ch_ts[1])
        e23 = smpool.tile([128, 128], F32)
        nc.vector.tensor_add(e23, ch_ts[2], ch_ts[3])
        e128 = smpool.tile([128, 128], F32)
        nc.vector.tensor_add(e128, e01, e23)
        mx = smpool.tile([128, 1], F32)
        nc.vector.reduce_max(mx, e128, axis=mybir.AxisListType.X)
        mxa = smpool.tile([128, 1], F32)
        nc.gpsimd.partition_all_reduce(
            mxa, mx, channels=128, reduce_op=bass_isa.ReduceOp.max
        )
        nmx = smpool.tile([128, 1], F32)
        nc.vector.tensor_scalar_mul(nmx, mxa, -1.0)
        ex = smpool.tile([128, 128], F32)
        nc.scalar.activation(ex, e128, AF.Exp, bias=nmx, scale=1.0)
        ee = smpool.tile([128, 128], F32)
        nc.vector.tensor_mul(ee, ex, im128)
        sm = smpool.tile([128, 1], F32)
        nc.vector.reduce_sum(sm, ee, axis=mybir.AxisListType.X)
        sma = smpool.tile([128, 1], F32)
        nc.gpsimd.partition_all_reduce(
            sma, sm, channels=128, reduce_op=bass_isa.ReduceOp.add
        )
        smb = smpool.tile([128, 1], F32)
        nc.vector.tensor_scalar_add(smb, sma, 1e-10)
        rr = smpool.tile([128, 1], F32)
        nc.vector.reciprocal(rr, smb)
        al = smpool.tile([128, 128], F32)
        nc.vector.tensor_scalar_mul(al, ee, rr)
        nas = smpool.tile([128, 128], F32)
        nc.vector.tensor_add(nas, al, am128)
        nc.sync.dma_start(out=alpha_d.rearrange("(p f) -> p f", p=128), in_=al)
        nc.sync.dma_start(out=nas_d.rearrange("(p f) -> p f", p=128), in_=nas)

    nc.compile()
    return nc


def _build_ctx_nc():
    """Fallback dense context matvec: ctx[c] = sum_hw wv[hw] * feat[c, hw]."""
    nc = bacc.Bacc("TRN2", target_bir_lowering=False, debug=False, num_devices=B)
    feat_d = nc.dram_tensor("feat", [C, HW], F32, kind="ExternalInput").ap()
    wv_d = nc.dram_tensor("wv", [HW], F32, kind="ExternalInput").ap()
    ctx_d = nc.dram_tensor("ctx_o", [C], F32, kind="ExternalOutput").ap()

    ncc = (C + 127) // 128  # 6 chunks (5 x 128 + 44)
    nhw = 8                 # hw chunks of 2048

    with tile.TileContext(nc) as tc, ExitStack() as ctx:
        pool = ctx.enter_context(tc.tile_pool(name="p", bufs=3))
        accp = ctx.enter_context(tc.tile_pool(name="acc", bufs=1))
        acc = [accp.tile([128, ncc], F32, name=f"acc{i}", tag=f"acc{i}")
               for i in range(2)]
        nc.vector.memset(acc[0], 0.0)
        nc.vector.memset(acc[1], 0.0)
        for ih in range(nhw):
            wv_sb = pool.tile([1, TBIG], F32, tag="wv")
            nc.sync.dma_start(out=wv_sb, in_=wv_d[ih * TBIG:(ih + 1) * TBIG][None, :])
            wrep = pool.tile([128, TBIG], F32, tag="wrep")
            nc.gpsimd.partition_broadcast(wrep, wv_sb, channels=128)
            src, dst = acc[ih % 2], acc[(ih + 1) % 2]
            for cc in range(ncc):
                csz = min(128, C - cc * 128)
                ft = pool.tile([128, TBIG], F32, tag="ft")
                nc.sync.dma_start(
                    out=ft[:csz],
                    in_=feat_d[cc * 128:cc * 128 + csz, ih * TBIG:(ih + 1) * TBIG],
                )
                junk = pool.tile([128, TBIG], F32, tag="junk")
                nc.vector.tensor_tensor_reduce(
                    out=junk[:csz],
                    in0=ft[:csz],
                    in1=wrep[:csz],
                    scale=1.0,
                    scalar=src[:csz, cc:cc + 1],
                    op0=mybir.AluOpType.mult,
                    op1=mybir.AluOpType.add,
                    accum_out=dst[:csz, cc:cc + 1],
                )
        fin = acc[nhw % 2]
        for cc in range(ncc):
            csz = min(128, C - cc * 128)
            nc.sync.dma_start(
                out=ctx_d[cc * 128:cc * 128 + csz][:, None], in_=fin[:csz, cc:cc + 1]
            )
    nc.compile()
    return nc


_CACHE = {}


def _get_nc(name, builder):
    if name not in _CACHE:
        _CACHE[name] = builder()
    return _CACHE[name]


def _host_prep(cnn_features_trans, hidden, alpha_sum, image_mask,
               W_h, b_h, conv_w, W_att, W_alpha):
    f32 = np.float32
    query = (hidden.astype(f32) @ W_h.astype(f32) + b_h.astype(f32))  # [8, 512]
    Mf = np.ascontiguousarray(
        np.einsum("cij,ca->ija", conv_w[:, 0].astype(f32), W_att.astype(f32))
        .reshape(TAPS, A)
    )
    apad = np.pad(alpha_sum[:, 0].astype(f32),
                  ((0, 0), (KS // 2, KS // 2), (KS // 2, KS // 2)))
    from numpy.lib.stride_tricks import sliding_window_view
    sw = sliding_window_view(apad, (KS, KS), axis=(1, 2))  # [B, H, W, 11, 11]
    import ml_dtypes
    X8 = np.ascontiguousarray(
        sw.transpose(0, 3, 4, 1, 2).reshape(B, TAPS, HW)
    ).astype(ml_dtypes.bfloat16)
    qp = np.ascontiguousarray(query.reshape(B, NCH, 128).transpose(0, 2, 1))
    Mf = Mf.astype(ml_dtypes.bfloat16)
    wa = np.zeros((128, NCH * 32), np.float32)
    wa[:, 0::32] = W_alpha[:, 0].astype(f32).reshape(NCH, 128).T
    wa = wa.astype(ml_dtypes.bfloat16)
    id128 = np.eye(128, dtype=ml_dtypes.bfloat16)
    tr = np.ascontiguousarray(cnn_features_trans.astype(f32).reshape(B, A, HW)).astype(ml_dtypes.bfloat16)
    am = np.ascontiguousarray(alpha_sum.astype(f32).reshape(B, HW))
    im = np.ascontiguousarray(image_mask.astype(f32).reshape(B, HW))
    return X8, tr, qp, Mf, wa, am, im, id128


def kernel(cnn_features, cnn_features_trans, hidden, alpha_sum, image_mask,
           W_h, b_h, conv_w, W_att, W_alpha, b_alpha):
    cnn_features = np.asarray(cnn_features)
    args = [np.asarray(a) for a in (cnn_features_trans, hidden, alpha_sum,
                                    image_mask, W_h, b_h, conv_w, W_att,
                                    W_alpha)]
    X8, tr, qp, Mf, wa, am, im, id128 = _host_prep(*args)

    nc = _get_nc("main", _build_main_nc)
    in_maps = [
        {"xt": X8[b], "tr": tr[b], "qp": qp[b], "mm": Mf, "wa": wa,
         "am": am[b], "im": im[b], "id128": id128}
        for b in range(B)
    ]
    res = bass_utils.run_bass_kernel_spmd(nc, in_maps, core_ids=list(range(B)))
    alpha = np.stack([res.results[b]["alpha_o"] for b in range(B)]).reshape(B, H, W)
    nas = np.stack([res.results[b]["nas_o"] for b in range(B)]).reshape(B, 1, H, W)

    # context_vector: alpha > 0.02 can hold at <= 49 positions (sum(alpha) <= 1);
    # for generic inputs it never holds and the exact result is 0.
    mask = alpha > np.float32(0.02)
    if mask.any():
        wv = (alpha * mask).reshape(B, HW).astype(np.float32)
        feat = np.ascontiguousarray(cnn_features.astype(np.float32).reshape(B, C, HW))
        ncc = _get_nc("ctx", _build_ctx_nc)
        in_maps2 = [{"feat": feat[b], "wv": wv[b]} for b in range(B)]
        res2 = bass_utils.run_bass_kernel_spmd(ncc, in_maps2, core_ids=list(range(B)))
        ctxv = np.stack([res2.results[b]["ctx_o"] for b in range(B)])
    else:
        ctxv = np.zeros((B, C), np.float32)

    return ctxv, alpha.astype(np.float32), nas.astype(np.float32)


# revision 17
# speedup vs baseline: 1.0375x; 1.0375x over previous
"""Trainium2 Bass kernel for the coverage-attention module.

Strategy (data-parallel over batch B=8, one batch per NeuronCore):
  - Host precompute (cheap, layout-only / tiny GEMMs):
      * query = hidden @ W_h + b_h                      [8, 512]
      * M     = einsum(conv_w, W_att) -> [121, 512]     (fuses the 11x11 conv
        with the channel-mixing einsum: coverage_alpha = im2col(alpha_sum) @ M)
      * X     = im2col of padded alpha_sum -> [8, 121, 16384]
  - Device (per core): conv-as-matmul + trans add + tanh + energy matvec on
    the tensor engine, then a local softmax (the global max subtraction is
    numerically equivalent to a per-sample max shift: softmax is shift
    invariant and the 1e-10 denominator guard perturbs at ~1e-13 relative,
    far below fp32 resolution).
  - context_vector = einsum(alpha * (alpha > 0.02), cnn_features).  Since
    sum(alpha) <= 1, at most 49 positions can exceed 0.02; for generic
    inputs none do and the result is exactly 0.  The mask is checked on the
    host from the returned alpha; only if any position passes is a second
    (dense matvec) kernel built and run.
"""

import numpy as np
from contextlib import ExitStack

import concourse.bass as bass
import concourse.tile as tile
from concourse import bacc, mybir, bass_isa
from concourse import bass_utils

B, C, H, W = 8, 684, 64, 256
HID, A, CONV_CH, KS = 256, 512, 512, 11
HW = H * W              # 16384
TAPS = KS * KS          # 121
NCH = A // 128          # 4 chunks of the attention dim
NT4 = 8                 # big DMA tiles along hw (2048 wide)
NTT = 4                 # compute subtiles per big tile (512 wide)
TBIG = HW // NT4        # 2048
TSUB = TBIG // NTT      # 512
F32 = mybir.dt.float32
BF16 = mybir.dt.bfloat16
AF = mybir.ActivationFunctionType
USE_PE_FRAC = False


def _build_main_nc():
    nc = bacc.Bacc("TRN2", target_bir_lowering=False, debug=False, num_devices=B)

    xt_d = nc.dram_tensor("xt", [TAPS, HW], BF16, kind="ExternalInput").ap()
    tr_d = nc.dram_tensor("tr", [A, HW], BF16, kind="ExternalInput").ap()
    qp_d = nc.dram_tensor("qp", [128, NCH], F32, kind="ExternalInput").ap()
    mm_d = nc.dram_tensor("mm", [TAPS, A], BF16, kind="ExternalInput").ap()
    wa_d = nc.dram_tensor("wa", [128, NCH * 32], BF16, kind="ExternalInput").ap()
    id_d = nc.dram_tensor("id128", [128, 128], BF16, kind="ExternalInput").ap()
    am_d = nc.dram_tensor("am", [HW], F32, kind="ExternalInput").ap()
    im_d = nc.dram_tensor("im", [HW], F32, kind="ExternalInput").ap()
    alpha_d = nc.dram_tensor("alpha_o", [HW], F32, kind="ExternalOutput").ap()
    nas_d = nc.dram_tensor("nas_o", [HW], F32, kind="ExternalOutput").ap()
    e_dram = nc.dram_tensor("e_scratch", [NCH, HW], F32, kind="Internal").ap()

    with tile.TileContext(nc) as tc, ExitStack() as ctx:
        consts = ctx.enter_context(tc.tile_pool(name="consts", bufs=1))
        xpool = ctx.enter_context(tc.tile_pool(name="xpool", bufs=3))
        trpool = ctx.enter_context(tc.tile_pool(name="trpool", bufs=16))
        prepool = ctx.enter_context(tc.tile_pool(name="prepool", bufs=3))
        scpool = ctx.enter_context(tc.tile_pool(name="scpool", bufs=6))
        evpool = ctx.enter_context(tc.tile_pool(name="evpool", bufs=2))
        smpool = ctx.enter_context(tc.tile_pool(name="smpool", bufs=1))
        cvps = ctx.enter_context(tc.tile_pool(name="cvps", bufs=3, space="PSUM"))
        epps = ctx.enter_context(tc.tile_pool(name="epps", bufs=2, space="PSUM"))

        mm_sb = consts.tile([TAPS, A], BF16)
        nc.sync.dma_start(out=mm_sb, in_=mm_d)
        qp_sb = consts.tile([128, NCH], F32)
        nc.sync.dma_start(out=qp_sb, in_=qp_d)
        wa_sb = consts.tile([128, NCH * 32], BF16)
        nc.sync.dma_start(out=wa_sb, in_=wa_d)
        id_sb = consts.tile([128, 128], BF16)
        nc.sync.dma_start(out=id_sb, in_=id_d)
        im128 = consts.tile([128, 128], F32)
        nc.sync.dma_start(out=im128, in_=im_d.rearrange("(p f) -> p f", p=128))
        am128 = consts.tile([128, 128], F32)
        nc.sync.dma_start(out=am128, in_=am_d.rearrange("(p f) -> p f", p=128))

        for t4 in range(NT4):
            xt_t = xpool.tile([TAPS, TBIG], BF16, tag="xt")
            nc.sync.dma_start(out=xt_t, in_=xt_d[:, t4 * TBIG:(t4 + 1) * TBIG])
            tr_ts = []
            for c in range(NCH):
                tr_t = trpool.tile([128, TBIG], BF16, tag="tr")
                nc.sync.dma_start(
                    out=tr_t,
                    in_=tr_d[c * 128:(c + 1) * 128, t4 * TBIG:(t4 + 1) * TBIG],
                )
                tr_ts.append(tr_t)
            # sc_ap(c, tt) -> AP of the [128, 512] tanh(score) slice
            sc_aps = {}
            for c in range(NCH):
                use_pe = USE_PE_FRAC and (t4 * NCH + c) % 2 == 0
                if use_pe:
                    # trans-add on the tensor engine (identity accumulate),
                    # tanh straight out of PSUM
                    for j in range(TBIG // 1024):
                        lo = j * 1024
                        cv = cvps.tile([128, 1024], F32, tag="cv")
                        for s in range(2):
                            nc.tensor.matmul(
                                cv[:, s * TSUB:(s + 1) * TSUB],
                                mm_sb[:, c * 128:(c + 1) * 128],
                                xt_t[:, lo + s * TSUB:lo + (s + 1) * TSUB],
                                start=True,
                                stop=True,
                            )
                        for s in range(2):
                            nc.tensor.matmul(
                                cv[:, s * TSUB:(s + 1) * TSUB],
                                id_sb,
                                tr_ts[c][:, lo + s * TSUB:lo + (s + 1) * TSUB],
                                start=False,
                                stop=True,
                                skip_group_check=True,
                            )
                        sc1k = scpool.tile([128, 1024], BF16, tag="sc1k")
                        nc.scalar.activation(
                            sc1k, cv, AF.Tanh, bias=qp_sb[:, c:c + 1], scale=1.0
                        )
                        sc_aps[(c, 2 * j)] = sc1k[:, 0:TSUB]
                        sc_aps[(c, 2 * j + 1)] = sc1k[:, TSUB:1024]
                else:
                    # trans-add on the vector engine, tanh from SBUF
                    pre2k = prepool.tile([128, TBIG], F32, tag="pre")
                    for j in range(TBIG // 1024):
                        lo = j * 1024
                        cv = cvps.tile([128, 1024], F32, tag="cv")
                        for s in range(2):
                            nc.tensor.matmul(
                                cv[:, s * TSUB:(s + 1) * TSUB],
                                mm_sb[:, c * 128:(c + 1) * 128],
                                xt_t[:, lo + s * TSUB:lo + (s + 1) * TSUB],
                                start=True,
                                stop=True,
                            )
                        nc.vector.tensor_add(
                            pre2k[:, lo:lo + 1024], cv, tr_ts[c][:, lo:lo + 1024]
                        )
                    sc2k = scpool.tile([128, TBIG], BF16, tag="sc")
                    nc.scalar.activation(
                        sc2k, pre2k, AF.Tanh, bias=qp_sb[:, c:c + 1], scale=1.0
                    )
                    for tt in range(NTT):
                        sc_aps[(c, tt)] = sc2k[:, tt * TSUB:(tt + 1) * TSUB]
            for tt in range(NTT):
                t = t4 * NTT + tt
                ep = epps.tile([128, TSUB], F32, tag="ep")
                for c in range(NCH):
                    nc.tensor.matmul(
                        ep[32 * c:32 * (c + 1), :],
                        wa_sb[:, c * 32:(c + 1) * 32],
                        sc_aps[(c, tt)],
                        start=True,
                        stop=True,
                        tile_position=(0, 32 * c),
                    )
                ev = evpool.tile([97, TSUB], F32, tag="ev")
                nc.scalar.copy(ev, ep[0:97, :])
                nc.sync.dma_start(out=e_dram[:, t * TSUB:(t + 1) * TSUB],
                                  in_=ev[::32, :])

        # --- softmax over the 16384 positions, done on a [128, 128] layout ---
        ch_ts = []
        for c in range(NCH):
            ch_t = smpool.tile([128, 128], F32, name=f"ch{c}", tag=f"ch{c}")
            nc.sync.dma_start(out=ch_t,
                              in_=e_dram[c].rearrange("(p f) -> p f", p=128))
            ch_ts.append(ch_t)
        e01 = smpool.tile([128, 128], F32)
        nc.vector.tensor_add(e01, ch_ts[0], ch_ts[1])
        e23 = smpool.tile([128, 128], F32)
        nc.vector.tensor_add(e23, ch_ts[2], ch_ts[3])
        e128 = smpool.tile([128, 128], F32)
        nc.vector.tensor_add(e128, e01, e23)
        mx = smpool.tile([128, 1], F32)
        nc.vector.reduce_max(mx, e128, axis=mybir.AxisListType.X)
        mxa = smpool.tile([128, 1], F32)
        nc.gpsimd.partition_all_reduce(
            mxa, mx, channels=128, reduce_op=bass_isa.ReduceOp.max
        )
        nmx = smpool.tile([128, 1], F32)
        nc.vector.tensor_scalar_mul(nmx, mxa, -1.0)
        ex = smpool.tile([128, 128], F32)
        nc.scalar.activation(ex, e128, AF.Exp, bias=nmx, scale=1.0)
        ee = smpool.tile([128, 128], F32)
        nc.vector.tensor_mul(ee, ex, im128)
        sm = smpool.tile([128, 1], F32)
        nc.vector.reduce_sum(sm, ee, axis=mybir.AxisListType.X)
        sma = smpool.tile([128, 1], F32)
        nc.gpsimd.partition_all_reduce(
            sma, sm, channels=128, reduce_op=bass_isa.ReduceOp.add
        )
        smb = smpool.tile([128, 1], F32)
        nc.vector.tensor_scalar_add(smb, sma, 1e-10)
        rr = smpool.tile([128, 1], F32)
        nc.vector.reciprocal(rr, smb)
        al = smpool.tile([128, 128], F32)
        nc.vector.tensor_scalar_mul(al, ee, rr)
        nas = smpool.tile([128, 128], F32)
        nc.vector.tensor_add(nas, al, am128)
        nc.sync.dma_start(out=alpha_d.rearrange("(p f) -> p f", p=128), in_=al)
        nc.sync.dma_start(out=nas_d.rearrange("(p f) -> p f", p=128), in_=nas)

    nc.compile()
    return nc


def _build_ctx_nc():
    """Fallback dense context matvec: ctx[c] = sum_hw wv[hw] * feat[c, hw]."""
    nc = bacc.Bacc("TRN2", target_bir_lowering=False, debug=False, num_devices=B)
    feat_d = nc.dram_tensor("feat", [C, HW], F32, kind="ExternalInput").ap()
    wv_d = nc.dram_tensor("wv", [HW], F32, kind="ExternalInput").ap()
    ctx_d = nc.dram_tensor("ctx_o", [C], F32, kind="ExternalOutput").ap()

    ncc = (C + 127) // 128  # 6 chunks (5 x 128 + 44)
    nhw = 8                 # hw chunks of 2048

    with tile.TileContext(nc) as tc, ExitStack() as ctx:
        pool = ctx.enter_context(tc.tile_pool(name="p", bufs=3))
        accp = ctx.enter_context(tc.tile_pool(name="acc", bufs=1))
        acc = [accp.tile([128, ncc], F32, name=f"acc{i}", tag=f"acc{i}")
               for i in range(2)]
        nc.vector.memset(acc[0], 0.0)
        nc.vector.memset(acc[1], 0.0)
        for ih in range(nhw):
            wv_sb = pool.tile([1, TBIG], F32, tag="wv")
            nc.sync.dma_start(out=wv_sb, in_=wv_d[ih * TBIG:(ih + 1) * TBIG][None, :])
            wrep = pool.tile([128, TBIG], F32, tag="wrep")
            nc.gpsimd.partition_broadcast(wrep, wv_sb, channels=128)
            src, dst = acc[ih % 2], acc[(ih + 1) % 2]
            for cc in range(ncc):
                csz = min(128, C - cc * 128)
                ft = pool.tile([128, TBIG], F32, tag="ft")
                nc.sync.dma_start(
                    out=ft[:csz],
                    in_=feat_d[cc * 128:cc * 128 + csz, ih * TBIG:(ih + 1) * TBIG],
                )
                junk = pool.tile([128, TBIG], F32, tag="junk")
                nc.vector.tensor_tensor_reduce(
                    out=junk[:csz],
                    in0=ft[:csz],
                    in1=wrep[:csz],
                    scale=1.0,
                    scalar=src[:csz, cc:cc + 1],
                    op0=mybir.AluOpType.mult,
                    op1=mybir.AluOpType.add,
                    accum_out=dst[:csz, cc:cc + 1],
                )
        fin = acc[nhw % 2]
        for cc in range(ncc):
            csz = min(128, C - cc * 128)
            nc.sync.dma_start(
                out=ctx_d[cc * 128:cc * 128 + csz][:, None], in_=fin[:csz, cc:cc + 1]
            )
    nc.compile()
    return nc


_CACHE = {}


def _get_nc(name, builder):
    if name not in _CACHE:
        _CACHE[name] = builder()
    return _CACHE[name]


def _host_prep(cnn_features_trans, hidden, alpha_sum, image_mask,
               W_h, b_h, conv_w, W_att, W_alpha):
    f32 = np.float32
    query = (hidden.astype(f32) @ W_h.astype(f32) + b_h.astype(f32))  # [8, 512]
    Mf = np.ascontiguousarray(
        np.einsum("cij,ca->ija", conv_w[:, 0].astype(f32), W_att.astype(f32))
        .reshape(TAPS, A)
    )
    apad = np.pad(alpha_sum[:, 0].astype(f32),
                  ((0, 0), (KS // 2, KS // 2), (KS // 2, KS // 2)))
    from numpy.lib.stride_tricks import sliding_window_view
    sw = sliding_window_view(apad, (KS, KS), axis=(1, 2))  # [B, H, W, 11, 11]
    import ml_dtypes
    X8 = np.ascontiguousarray(
        sw.transpose(0, 3, 4, 1, 2).reshape(B, TAPS, HW)
    ).astype(ml_dtypes.bfloat16)
    qp = np.ascontiguousarray(query.reshape(B, NCH, 128).transpose(0, 2, 1))
    Mf = Mf.astype(ml_dtypes.bfloat16)
    wa = np.zeros((128, NCH * 32), np.float32)
    wa[:, 0::32] = W_alpha[:, 0].astype(f32).reshape(NCH, 128).T
    wa = wa.astype(ml_dtypes.bfloat16)
    id128 = np.eye(128, dtype=ml_dtypes.bfloat16)
    tr = np.ascontiguousarray(cnn_features_trans.astype(f32).reshape(B, A, HW)).astype(ml_dtypes.bfloat16)
    am = np.ascontiguousarray(alpha_sum.astype(f32).reshape(B, HW))
    im = np.ascontiguousarray(image_mask.astype(f32).reshape(B, HW))
    return X8, tr, qp, Mf, wa, am, im, id128


def kernel(cnn_features, cnn_features_trans, hidden, alpha_sum, image_mask,
           W_h, b_h, conv_w, W_att, W_alpha, b_alpha):
    cnn_features = np.asarray(cnn_features)
    args = [np.asarray(a) for a in (cnn_features_trans, hidden, alpha_sum,
                                    image_mask, W_h, b_h, conv_w, W_att,
                                    W_alpha)]
    X8, tr, qp, Mf, wa, am, im, id128 = _host_prep(*args)

    nc = _get_nc("main", _build_main_nc)
    in_maps = [
        {"xt": X8[b], "tr": tr[b], "qp": qp[b], "mm": Mf, "wa": wa,
         "am": am[b], "im": im[b], "id128": id128}
        for b in range(B)
    ]
    res = bass_utils.run_bass_kernel_spmd(nc, in_maps, core_ids=list(range(B)))
    alpha = np.stack([res.results[b]["alpha_o"] for b in range(B)]).reshape(B, H, W)
    nas = np.stack([res.results[b]["nas_o"] for b in range(B)]).reshape(B, 1, H, W)

    # context_vector: alpha > 0.02 can hold at <= 49 positions (sum(alpha) <= 1);
    # for generic inputs it never holds and the exact result is 0.
    mask = alpha > np.float32(0.02)
    if mask.any():
        wv = (alpha * mask).reshape(B, HW).astype(np.float32)
        feat = np.ascontiguousarray(cnn_features.astype(np.float32).reshape(B, C, HW))
        ncc = _get_nc("ctx", _build_ctx_nc)
        in_maps2 = [{"feat": feat[b], "wv": wv[b]} for b in range(B)]
        res2 = bass_utils.run_bass_kernel_spmd(ncc, in_maps2, core_ids=list(range(B)))
        ctxv = np.stack([res2.results[b]["ctx_o"] for b in range(B)])
    else:
        ctxv = np.zeros((B, C), np.float32)

    return ctxv, alpha.astype(np.float32), nas.astype(np.float32)


# revision 18
# speedup vs baseline: 1.0639x; 1.0254x over previous
"""Trainium2 Bass kernel for the coverage-attention module.

Strategy (data-parallel over batch B=8, one batch per NeuronCore):
  - Host precompute (cheap, layout-only / tiny GEMMs):
      * query = hidden @ W_h + b_h                      [8, 512]
      * M     = einsum(conv_w, W_att) -> [121, 512]     (fuses the 11x11 conv
        with the channel-mixing einsum: coverage_alpha = im2col(alpha_sum) @ M)
      * X     = im2col of padded alpha_sum -> [8, 121, 16384]
  - Device (per core): conv-as-matmul + trans add + tanh + energy matvec on
    the tensor engine, then a local softmax (the global max subtraction is
    numerically equivalent to a per-sample max shift: softmax is shift
    invariant and the 1e-10 denominator guard perturbs at ~1e-13 relative,
    far below fp32 resolution).
  - context_vector = einsum(alpha * (alpha > 0.02), cnn_features).  Since
    sum(alpha) <= 1, at most 49 positions can exceed 0.02; for generic
    inputs none do and the result is exactly 0.  The mask is checked on the
    host from the returned alpha; only if any position passes is a second
    (dense matvec) kernel built and run.
"""

import numpy as np
from contextlib import ExitStack

import concourse.bass as bass
import concourse.tile as tile
from concourse import bacc, mybir, bass_isa
from concourse import bass_utils

B, C, H, W = 8, 684, 64, 256
HID, A, CONV_CH, KS = 256, 512, 512, 11
HW = H * W              # 16384
TAPS = KS * KS          # 121
NCH = A // 128          # 4 chunks of the attention dim
NT4 = 8                 # big DMA tiles along hw (2048 wide)
NTT = 4                 # compute subtiles per big tile (512 wide)
TBIG = HW // NT4        # 2048
TSUB = TBIG // NTT      # 512
F32 = mybir.dt.float32
BF16 = mybir.dt.bfloat16
AF = mybir.ActivationFunctionType
USE_PE_FRAC = True


def _build_main_nc():
    nc = bacc.Bacc("TRN2", target_bir_lowering=False, debug=False, num_devices=B)

    xt_d = nc.dram_tensor("xt", [TAPS, HW], BF16, kind="ExternalInput").ap()
    tr_d = nc.dram_tensor("tr", [A, HW], BF16, kind="ExternalInput").ap()
    qp_d = nc.dram_tensor("qp", [128, NCH], F32, kind="ExternalInput").ap()
    mm_d = nc.dram_tensor("mm", [TAPS, A], BF16, kind="ExternalInput").ap()
    wa_d = nc.dram_tensor("wa", [128, NCH * 32], BF16, kind="ExternalInput").ap()
    id_d = nc.dram_tensor("id128", [128, 128], BF16, kind="ExternalInput").ap()
    am_d = nc.dram_tensor("am", [HW], F32, kind="ExternalInput").ap()
    im_d = nc.dram_tensor("im", [HW], F32, kind="ExternalInput").ap()
    alpha_d = nc.dram_tensor("alpha_o", [HW], F32, kind="ExternalOutput").ap()
    nas_d = nc.dram_tensor("nas_o", [HW], F32, kind="ExternalOutput").ap()
    e_dram = nc.dram_tensor("e_scratch", [NCH, HW], F32, kind="Internal").ap()

    with tile.TileContext(nc) as tc, ExitStack() as ctx:
        consts = ctx.enter_context(tc.tile_pool(name="consts", bufs=1))
        xpool = ctx.enter_context(tc.tile_pool(name="xpool", bufs=3))
        trpool = ctx.enter_context(tc.tile_pool(name="trpool", bufs=16))
        prepool = ctx.enter_context(tc.tile_pool(name="prepool", bufs=3))
        scpool = ctx.enter_context(tc.tile_pool(name="scpool", bufs=6))
        evpool = ctx.enter_context(tc.tile_pool(name="evpool", bufs=2))
        smpool = ctx.enter_context(tc.tile_pool(name="smpool", bufs=1))
        cvps = ctx.enter_context(tc.tile_pool(name="cvps", bufs=3, space="PSUM"))
        epps = ctx.enter_context(tc.tile_pool(name="epps", bufs=2, space="PSUM"))

        mm_sb = consts.tile([TAPS, A], BF16)
        nc.sync.dma_start(out=mm_sb, in_=mm_d)
        qp_sb = consts.tile([128, NCH], F32)
        nc.sync.dma_start(out=qp_sb, in_=qp_d)
        wa_sb = consts.tile([128, NCH * 32], BF16)
        nc.sync.dma_start(out=wa_sb, in_=wa_d)
        id_sb = consts.tile([128, 128], BF16)
        nc.sync.dma_start(out=id_sb, in_=id_d)
        im128 = consts.tile([128, 128], F32)
        nc.sync.dma_start(out=im128, in_=im_d.rearrange("(p f) -> p f", p=128))
        am128 = consts.tile([128, 128], F32)
        nc.sync.dma_start(out=am128, in_=am_d.rearrange("(p f) -> p f", p=128))

        for t4 in range(NT4):
            xt_t = xpool.tile([TAPS, TBIG], BF16, tag="xt")
            nc.sync.dma_start(out=xt_t, in_=xt_d[:, t4 * TBIG:(t4 + 1) * TBIG])
            tr_ts = []
            for c in range(NCH):
                tr_t = trpool.tile([128, TBIG], BF16, tag="tr")
                nc.sync.dma_start(
                    out=tr_t,
                    in_=tr_d[c * 128:(c + 1) * 128, t4 * TBIG:(t4 + 1) * TBIG],
                )
                tr_ts.append(tr_t)
            # sc_ap(c, tt) -> AP of the [128, 512] tanh(score) slice
            sc_aps = {}
            for c in range(NCH):
                use_pe = USE_PE_FRAC and (t4 * NCH + c) % 2 == 0
                if use_pe:
                    # trans-add on the tensor engine (identity accumulate),
                    # tanh straight out of PSUM
                    for j in range(TBIG // 1024):
                        lo = j * 1024
                        cv = cvps.tile([128, 1024], F32, tag="cv")
                        for s in range(2):
                            nc.tensor.matmul(
                                cv[:, s * TSUB:(s + 1) * TSUB],
                                mm_sb[:, c * 128:(c + 1) * 128],
                                xt_t[:, lo + s * TSUB:lo + (s + 1) * TSUB],
                                start=True,
                                stop=True,
                            )
                        for s in range(2):
                            nc.tensor.matmul(
                                cv[:, s * TSUB:(s + 1) * TSUB],
                                id_sb,
                                tr_ts[c][:, lo + s * TSUB:lo + (s + 1) * TSUB],
                                start=False,
                                stop=True,
                                skip_group_check=True,
                            )
                        sc1k = scpool.tile([128, 1024], BF16, tag="sc1k")
                        nc.scalar.activation(
                            sc1k, cv, AF.Tanh, bias=qp_sb[:, c:c + 1], scale=1.0
                        )
                        sc_aps[(c, 2 * j)] = sc1k[:, 0:TSUB]
                        sc_aps[(c, 2 * j + 1)] = sc1k[:, TSUB:1024]
                else:
                    # trans-add on the vector engine, tanh from SBUF
                    pre2k = prepool.tile([128, TBIG], F32, tag="pre")
                    for j in range(TBIG // 1024):
                        lo = j * 1024
                        cv = cvps.tile([128, 1024], F32, tag="cv")
                        for s in range(2):
                            nc.tensor.matmul(
                                cv[:, s * TSUB:(s + 1) * TSUB],
                                mm_sb[:, c * 128:(c + 1) * 128],
                                xt_t[:, lo + s * TSUB:lo + (s + 1) * TSUB],
                                start=True,
                                stop=True,
                            )
                        nc.vector.tensor_add(
                            pre2k[:, lo:lo + 1024], cv, tr_ts[c][:, lo:lo + 1024]
                        )
                    sc2k = scpool.tile([128, TBIG], BF16, tag="sc")
                    nc.scalar.activation(
                        sc2k, pre2k, AF.Tanh, bias=qp_sb[:, c:c + 1], scale=1.0
                    )
                    for tt in range(NTT):
                        sc_aps[(c, tt)] = sc2k[:, tt * TSUB:(tt + 1) * TSUB]
            for tt in range(NTT):
                t = t4 * NTT + tt
                ep = epps.tile([128, TSUB], F32, tag="ep")
                for c in range(NCH):
                    nc.tensor.matmul(
                        ep[32 * c:32 * (c + 1), :],
                        wa_sb[:, c * 32:(c + 1) * 32],
                        sc_aps[(c, tt)],
                        start=True,
                        stop=True,
                        tile_position=(0, 32 * c),
                    )
                ev = evpool.tile([97, TSUB], F32, tag="ev")
                nc.scalar.copy(ev, ep[0:97, :])
                nc.sync.dma_start(out=e_dram[:, t * TSUB:(t + 1) * TSUB],
                                  in_=ev[::32, :])

        # --- softmax over the 16384 positions, done on a [128, 128] layout ---
        ch_ts = []
        for c in range(NCH):
            ch_t = smpool.tile([128, 128], F32, name=f"ch{c}", tag=f"ch{c}")
            nc.sync.dma_start(out=ch_t,
                              in_=e_dram[c].rearrange("(p f) -> p f", p=128))
            ch_ts.append(ch_t)
        e01 = smpool.tile([128, 128], F32)
        nc.vector.tensor_add(e01, ch_ts[0], ch_ts[1])
        e23 = smpool.tile([128, 128], F32)
        nc.vector.tensor_add(e23, ch_ts[2], ch_ts[3])
        e128 = smpool.tile([128, 128], F32)
        nc.vector.tensor_add(e128, e01, e23)
        mx = smpool.tile([128, 1], F32)
        nc.vector.reduce_max(mx, e128, axis=mybir.AxisListType.X)
        mxa = smpool.tile([128, 1], F32)
        nc.gpsimd.partition_all_reduce(
            mxa, mx, channels=128, reduce_op=bass_isa.ReduceOp.max
        )
        nmx = smpool.tile([128, 1], F32)
        nc.vector.tensor_scalar_mul(nmx, mxa, -1.0)
        ex = smpool.tile([128, 128], F32)
        nc.scalar.activation(ex, e128, AF.Exp, bias=nmx, scale=1.0)
        ee = smpool.tile([128, 128], F32)
        nc.vector.tensor_mul(ee, ex, im128)
        sm = smpool.tile([128, 1], F32)
        nc.vector.reduce_sum(sm, ee, axis=mybir.AxisListType.X)
        sma = smpool.tile([128, 1], F32)
        nc.gpsimd.partition_all_reduce(
            sma, sm, channels=128, reduce_op=bass_isa.ReduceOp.add
        )
        smb = smpool.tile([128, 1], F32)
        nc.vector.tensor_scalar_add(smb, sma, 1e-10)
        rr = smpool.tile([128, 1], F32)
        nc.vector.reciprocal(rr, smb)
        al = smpool.tile([128, 128], F32)
        nc.vector.tensor_scalar_mul(al, ee, rr)
        nas = smpool.tile([128, 128], F32)
        nc.vector.tensor_add(nas, al, am128)
        nc.sync.dma_start(out=alpha_d.rearrange("(p f) -> p f", p=128), in_=al)
        nc.sync.dma_start(out=nas_d.rearrange("(p f) -> p f", p=128), in_=nas)

    nc.compile()
    return nc


def _build_ctx_nc():
    """Fallback dense context matvec: ctx[c] = sum_hw wv[hw] * feat[c, hw]."""
    nc = bacc.Bacc("TRN2", target_bir_lowering=False, debug=False, num_devices=B)
    feat_d = nc.dram_tensor("feat", [C, HW], F32, kind="ExternalInput").ap()
    wv_d = nc.dram_tensor("wv", [HW], F32, kind="ExternalInput").ap()
    ctx_d = nc.dram_tensor("ctx_o", [C], F32, kind="ExternalOutput").ap()

    ncc = (C + 127) // 128  # 6 chunks (5 x 128 + 44)
    nhw = 8                 # hw chunks of 2048

    with tile.TileContext(nc) as tc, ExitStack() as ctx:
        pool = ctx.enter_context(tc.tile_pool(name="p", bufs=3))
        accp = ctx.enter_context(tc.tile_pool(name="acc", bufs=1))
        acc = [accp.tile([128, ncc], F32, name=f"acc{i}", tag=f"acc{i}")
               for i in range(2)]
        nc.vector.memset(acc[0], 0.0)
        nc.vector.memset(acc[1], 0.0)
        for ih in range(nhw):
            wv_sb = pool.tile([1, TBIG], F32, tag="wv")
            nc.sync.dma_start(out=wv_sb, in_=wv_d[ih * TBIG:(ih + 1) * TBIG][None, :])
            wrep = pool.tile([128, TBIG], F32, tag="wrep")
            nc.gpsimd.partition_broadcast(wrep, wv_sb, channels=128)
            src, dst = acc[ih % 2], acc[(ih + 1) % 2]
            for cc in range(ncc):
                csz = min(128, C - cc * 128)
                ft = pool.tile([128, TBIG], F32, tag="ft")
                nc.sync.dma_start(
                    out=ft[:csz],
                    in_=feat_d[cc * 128:cc * 128 + csz, ih * TBIG:(ih + 1) * TBIG],
                )
                junk = pool.tile([128, TBIG], F32, tag="junk")
                nc.vector.tensor_tensor_reduce(
                    out=junk[:csz],
                    in0=ft[:csz],
                    in1=wrep[:csz],
                    scale=1.0,
                    scalar=src[:csz, cc:cc + 1],
                    op0=mybir.AluOpType.mult,
                    op1=mybir.AluOpType.add,
                    accum_out=dst[:csz, cc:cc + 1],
                )
        fin = acc[nhw % 2]
        for cc in range(ncc):
            csz = min(128, C - cc * 128)
            nc.sync.dma_start(
                out=ctx_d[cc * 128:cc * 128 + csz][:, None], in_=fin[:csz, cc:cc + 1]
            )
    nc.compile()
    return nc


_CACHE = {}


def _get_nc(name, builder):
    if name not in _CACHE:
        _CACHE[name] = builder()
    return _CACHE[name]


def _host_prep(cnn_features_trans, hidden, alpha_sum, image_mask,
               W_h, b_h, conv_w, W_att, W_alpha):
    f32 = np.float32
    query = (hidden.astype(f32) @ W_h.astype(f32) + b_h.astype(f32))  # [8, 512]
    Mf = np.ascontiguousarray(
        np.einsum("cij,ca->ija", conv_w[:, 0].astype(f32), W_att.astype(f32))
        .reshape(TAPS, A)
    )
    apad = np.pad(alpha_sum[:, 0].astype(f32),
                  ((0, 0), (KS // 2, KS // 2), (KS // 2, KS // 2)))
    from numpy.lib.stride_tricks import sliding_window_view
    sw = sliding_window_view(apad, (KS, KS), axis=(1, 2))  # [B, H, W, 11, 11]
    import ml_dtypes
    X8 = np.ascontiguousarray(
        sw.transpose(0, 3, 4, 1, 2).reshape(B, TAPS, HW)
    ).astype(ml_dtypes.bfloat16)
    qp = np.ascontiguousarray(query.reshape(B, NCH, 128).transpose(0, 2, 1))
    Mf = Mf.astype(ml_dtypes.bfloat16)
    wa = np.zeros((128, NCH * 32), np.float32)
    wa[:, 0::32] = W_alpha[:, 0].astype(f32).reshape(NCH, 128).T
    wa = wa.astype(ml_dtypes.bfloat16)
    id128 = np.eye(128, dtype=ml_dtypes.bfloat16)
    tr = np.ascontiguousarray(cnn_features_trans.astype(f32).reshape(B, A, HW)).astype(ml_dtypes.bfloat16)
    am = np.ascontiguousarray(alpha_sum.astype(f32).reshape(B, HW))
    im = np.ascontiguousarray(image_mask.astype(f32).reshape(B, HW))
    return X8, tr, qp, Mf, wa, am, im, id128


def kernel(cnn_features, cnn_features_trans, hidden, alpha_sum, image_mask,
           W_h, b_h, conv_w, W_att, W_alpha, b_alpha):
    cnn_features = np.asarray(cnn_features)
    args = [np.asarray(a) for a in (cnn_features_trans, hidden, alpha_sum,
                                    image_mask, W_h, b_h, conv_w, W_att,
                                    W_alpha)]
    X8, tr, qp, Mf, wa, am, im, id128 = _host_prep(*args)

    nc = _get_nc("main", _build_main_nc)
    in_maps = [
        {"xt": X8[b], "tr": tr[b], "qp": qp[b], "mm": Mf, "wa": wa,
         "am": am[b], "im": im[b], "id128": id128}
        for b in range(B)
    ]
    res = bass_utils.run_bass_kernel_spmd(nc, in_maps, core_ids=list(range(B)))
    alpha = np.stack([res.results[b]["alpha_o"] for b in range(B)]).reshape(B, H, W)
    nas = np.stack([res.results[b]["nas_o"] for b in range(B)]).reshape(B, 1, H, W)

    # context_vector: alpha > 0.02 can hold at <= 49 positions (sum(alpha) <= 1);
    # for generic inputs it never holds and the exact result is 0.
    mask = alpha > np.float32(0.02)
    if mask.any():
        wv = (alpha * mask).reshape(B, HW).astype(np.float32)
        feat = np.ascontiguousarray(cnn_features.astype(np.float32).reshape(B, C, HW))
        ncc = _get_nc("ctx", _build_ctx_nc)
        in_maps2 = [{"feat": feat[b], "wv": wv[b]} for b in range(B)]
        res2 = bass_utils.run_bass_kernel_spmd(ncc, in_maps2, core_ids=list(range(B)))
        ctxv = np.stack([res2.results[b]["ctx_o"] for b in range(B)])
    else:
        ctxv = np.zeros((B, C), np.float32)

    return ctxv, alpha.astype(np.float32), nas.astype(np.float32)


# revision 19
# speedup vs baseline: 1.0997x; 1.0336x over previous
"""Trainium2 Bass kernel for the coverage-attention module.

Strategy (data-parallel over batch B=8, one batch per NeuronCore):
  - Host precompute (cheap, layout-only / tiny GEMMs):
      * query = hidden @ W_h + b_h                      [8, 512]
      * M     = einsum(conv_w, W_att) -> [121, 512]     (fuses the 11x11 conv
        with the channel-mixing einsum: coverage_alpha = im2col(alpha_sum) @ M)
      * X     = im2col of padded alpha_sum -> [8, 121, 16384]
  - Device (per core): conv-as-matmul + trans add + tanh + energy matvec on
    the tensor engine, then a local softmax (the global max subtraction is
    numerically equivalent to a per-sample max shift: softmax is shift
    invariant and the 1e-10 denominator guard perturbs at ~1e-13 relative,
    far below fp32 resolution).
  - context_vector = einsum(alpha * (alpha > 0.02), cnn_features).  Since
    sum(alpha) <= 1, at most 49 positions can exceed 0.02; for generic
    inputs none do and the result is exactly 0.  The mask is checked on the
    host from the returned alpha; only if any position passes is a second
    (dense matvec) kernel built and run.
"""

import numpy as np
from contextlib import ExitStack

import concourse.bass as bass
import concourse.tile as tile
from concourse import bacc, mybir, bass_isa
from concourse import bass_utils

B, C, H, W = 8, 684, 64, 256
HID, A, CONV_CH, KS = 256, 512, 512, 11
HW = H * W              # 16384
TAPS = KS * KS          # 121
TAPSP = 128             # taps padded to 128 for FWL (K=128)
NCH = A // 128          # 4 chunks of the attention dim
NT4 = 8                 # big DMA tiles along hw (2048 wide)
NTT = 4                 # compute subtiles per big tile (512 wide)
TBIG = HW // NT4        # 2048
TSUB = TBIG // NTT      # 512
F32 = mybir.dt.float32
BF16 = mybir.dt.bfloat16
AF = mybir.ActivationFunctionType
USE_PE_FRAC = True


def _build_main_nc():
    nc = bacc.Bacc("TRN2", target_bir_lowering=False, debug=False, num_devices=B)

    xt_d = nc.dram_tensor("xt", [TAPSP, HW], BF16, kind="ExternalInput").ap()
    tr_d = nc.dram_tensor("tr", [A, HW], BF16, kind="ExternalInput").ap()
    qp_d = nc.dram_tensor("qp", [128, NCH], F32, kind="ExternalInput").ap()
    mm_d = nc.dram_tensor("mm", [TAPSP, A], BF16, kind="ExternalInput").ap()
    wa_d = nc.dram_tensor("wa", [128, NCH * 32], BF16, kind="ExternalInput").ap()
    id_d = nc.dram_tensor("id128", [128, 128], BF16, kind="ExternalInput").ap()
    am_d = nc.dram_tensor("am", [HW], F32, kind="ExternalInput").ap()
    im_d = nc.dram_tensor("im", [HW], F32, kind="ExternalInput").ap()
    alpha_d = nc.dram_tensor("alpha_o", [HW], F32, kind="ExternalOutput").ap()
    nas_d = nc.dram_tensor("nas_o", [HW], F32, kind="ExternalOutput").ap()
    e_dram = nc.dram_tensor("e_scratch", [NCH, HW], F32, kind="Internal").ap()

    with tile.TileContext(nc) as tc, ExitStack() as ctx:
        consts = ctx.enter_context(tc.tile_pool(name="consts", bufs=1))
        xpool = ctx.enter_context(tc.tile_pool(name="xpool", bufs=3))
        trpool = ctx.enter_context(tc.tile_pool(name="trpool", bufs=16))
        prepool = ctx.enter_context(tc.tile_pool(name="prepool", bufs=3))
        scpool = ctx.enter_context(tc.tile_pool(name="scpool", bufs=6))
        evpool = ctx.enter_context(tc.tile_pool(name="evpool", bufs=2))
        smpool = ctx.enter_context(tc.tile_pool(name="smpool", bufs=1))
        cvps = ctx.enter_context(tc.tile_pool(name="cvps", bufs=3, space="PSUM"))
        epps = ctx.enter_context(tc.tile_pool(name="epps", bufs=2, space="PSUM"))

        mm_sb = consts.tile([TAPSP, A], BF16)
        nc.sync.dma_start(out=mm_sb, in_=mm_d)
        qp_sb = consts.tile([128, NCH], F32)
        nc.sync.dma_start(out=qp_sb, in_=qp_d)
        wa_sb = consts.tile([128, NCH * 32], BF16)
        nc.sync.dma_start(out=wa_sb, in_=wa_d)
        id_sb = consts.tile([128, 128], BF16)
        nc.sync.dma_start(out=id_sb, in_=id_d)
        im128 = consts.tile([128, 128], F32)
        nc.sync.dma_start(out=im128, in_=im_d.rearrange("(p f) -> p f", p=128))
        am128 = consts.tile([128, 128], F32)
        nc.sync.dma_start(out=am128, in_=am_d.rearrange("(p f) -> p f", p=128))

        for t4 in range(NT4):
            xt_t = xpool.tile([TAPSP, TBIG], BF16, tag="xt")
            nc.sync.dma_start(out=xt_t, in_=xt_d[:, t4 * TBIG:(t4 + 1) * TBIG])
            tr_ts = []
            for c in range(NCH):
                tr_t = trpool.tile([128, TBIG], BF16, tag="tr")
                nc.sync.dma_start(
                    out=tr_t,
                    in_=tr_d[c * 128:(c + 1) * 128, t4 * TBIG:(t4 + 1) * TBIG],
                )
                tr_ts.append(tr_t)
            # sc_ap(c, tt) -> AP of the [128, 512] tanh(score) slice
            sc_aps = {}
            for c in range(NCH):
                use_pe = USE_PE_FRAC and (t4 * NCH + c) % 2 == 0
                if use_pe:
                    # trans-add on the tensor engine (identity accumulate),
                    # tanh straight out of PSUM
                    for j in range(TBIG // 1024):
                        lo = j * 1024
                        cv = cvps.tile([128, 1024], F32, tag="cv")
                        for s in range(2):
                            nc.tensor.matmul(
                                cv[:, s * TSUB:(s + 1) * TSUB],
                                mm_sb[:, c * 128:(c + 1) * 128],
                                xt_t[:, lo + s * TSUB:lo + (s + 1) * TSUB],
                                start=True,
                                stop=True,
                            )
                        for s in range(2):
                            nc.tensor.matmul(
                                cv[:, s * TSUB:(s + 1) * TSUB],
                                id_sb,
                                tr_ts[c][:, lo + s * TSUB:lo + (s + 1) * TSUB],
                                start=False,
                                stop=True,
                                skip_group_check=True,
                            )
                        sc1k = scpool.tile([128, 1024], BF16, tag="sc1k")
                        nc.scalar.activation(
                            sc1k, cv, AF.Tanh, bias=qp_sb[:, c:c + 1], scale=1.0
                        )
                        sc_aps[(c, 2 * j)] = sc1k[:, 0:TSUB]
                        sc_aps[(c, 2 * j + 1)] = sc1k[:, TSUB:1024]
                else:
                    # trans-add on the vector engine, tanh from SBUF
                    pre2k = prepool.tile([128, TBIG], F32, tag="pre")
                    for j in range(TBIG // 1024):
                        lo = j * 1024
                        cv = cvps.tile([128, 1024], F32, tag="cv")
                        for s in range(2):
                            nc.tensor.matmul(
                                cv[:, s * TSUB:(s + 1) * TSUB],
                                mm_sb[:, c * 128:(c + 1) * 128],
                                xt_t[:, lo + s * TSUB:lo + (s + 1) * TSUB],
                                start=True,
                                stop=True,
                            )
                        nc.vector.tensor_add(
                            pre2k[:, lo:lo + 1024], cv, tr_ts[c][:, lo:lo + 1024]
                        )
                    sc2k = scpool.tile([128, TBIG], BF16, tag="sc")
                    nc.scalar.activation(
                        sc2k, pre2k, AF.Tanh, bias=qp_sb[:, c:c + 1], scale=1.0
                    )
                    for tt in range(NTT):
                        sc_aps[(c, tt)] = sc2k[:, tt * TSUB:(tt + 1) * TSUB]
            for tt in range(NTT):
                t = t4 * NTT + tt
                ep = epps.tile([128, TSUB], F32, tag="ep")
                for c in range(NCH):
                    nc.tensor.matmul(
                        ep[32 * c:32 * (c + 1), :],
                        wa_sb[:, c * 32:(c + 1) * 32],
                        sc_aps[(c, tt)],
                        start=True,
                        stop=True,
                        tile_position=(0, 32 * c),
                    )
                ev = evpool.tile([97, TSUB], F32, tag="ev")
                nc.vector.tensor_copy(ev, ep[0:97, :])
                nc.sync.dma_start(out=e_dram[:, t * TSUB:(t + 1) * TSUB],
                                  in_=ev[::32, :])

        # --- softmax over the 16384 positions, done on a [128, 128] layout ---
        ch_ts = []
        for c in range(NCH):
            ch_t = smpool.tile([128, 128], F32, name=f"ch{c}", tag=f"ch{c}")
            nc.sync.dma_start(out=ch_t,
                              in_=e_dram[c].rearrange("(p f) -> p f", p=128))
            ch_ts.append(ch_t)
        e01 = smpool.tile([128, 128], F32)
        nc.vector.tensor_add(e01, ch_ts[0], ch_ts[1])
        e23 = smpool.tile([128, 128], F32)
        nc.vector.tensor_add(e23, ch_ts[2], ch_ts[3])
        e128 = smpool.tile([128, 128], F32)
        nc.vector.tensor_add(e128, e01, e23)
        mx = smpool.tile([128, 1], F32)
        nc.vector.reduce_max(mx, e128, axis=mybir.AxisListType.X)
        mxa = smpool.tile([128, 1], F32)
        nc.gpsimd.partition_all_reduce(
            mxa, mx, channels=128, reduce_op=bass_isa.ReduceOp.max
        )
        nmx = smpool.tile([128, 1], F32)
        nc.vector.tensor_scalar_mul(nmx, mxa, -1.0)
        ex = smpool.tile([128, 128], F32)
        nc.scalar.activation(ex, e128, AF.Exp, bias=nmx, scale=1.0)
        ee = smpool.tile([128, 128], F32)
        nc.vector.tensor_mul(ee, ex, im128)
        sm = smpool.tile([128, 1], F32)
        nc.vector.reduce_sum(sm, ee, axis=mybir.AxisListType.X)
        sma = smpool.tile([128, 1], F32)
        nc.gpsimd.partition_all_reduce(
            sma, sm, channels=128, reduce_op=bass_isa.ReduceOp.add
        )
        smb = smpool.tile([128, 1], F32)
        nc.vector.tensor_scalar_add(smb, sma, 1e-10)
        rr = smpool.tile([128, 1], F32)
        nc.vector.reciprocal(rr, smb)
        al = smpool.tile([128, 128], F32)
        nc.vector.tensor_scalar_mul(al, ee, rr)
        nas = smpool.tile([128, 128], F32)
        nc.vector.tensor_add(nas, al, am128)
        nc.sync.dma_start(out=alpha_d.rearrange("(p f) -> p f", p=128), in_=al)
        nc.sync.dma_start(out=nas_d.rearrange("(p f) -> p f", p=128), in_=nas)

    nc.compile()
    return nc


def _build_ctx_nc():
    """Fallback dense context matvec: ctx[c] = sum_hw wv[hw] * feat[c, hw]."""
    nc = bacc.Bacc("TRN2", target_bir_lowering=False, debug=False, num_devices=B)
    feat_d = nc.dram_tensor("feat", [C, HW], F32, kind="ExternalInput").ap()
    wv_d = nc.dram_tensor("wv", [HW], F32, kind="ExternalInput").ap()
    ctx_d = nc.dram_tensor("ctx_o", [C], F32, kind="ExternalOutput").ap()

    ncc = (C + 127) // 128  # 6 chunks (5 x 128 + 44)
    nhw = 8                 # hw chunks of 2048

    with tile.TileContext(nc) as tc, ExitStack() as ctx:
        pool = ctx.enter_context(tc.tile_pool(name="p", bufs=3))
        accp = ctx.enter_context(tc.tile_pool(name="acc", bufs=1))
        acc = [accp.tile([128, ncc], F32, name=f"acc{i}", tag=f"acc{i}")
               for i in range(2)]
        nc.vector.memset(acc[0], 0.0)
        nc.vector.memset(acc[1], 0.0)
        for ih in range(nhw):
            wv_sb = pool.tile([1, TBIG], F32, tag="wv")
            nc.sync.dma_start(out=wv_sb, in_=wv_d[ih * TBIG:(ih + 1) * TBIG][None, :])
            wrep = pool.tile([128, TBIG], F32, tag="wrep")
            nc.gpsimd.partition_broadcast(wrep, wv_sb, channels=128)
            src, dst = acc[ih % 2], acc[(ih + 1) % 2]
            for cc in range(ncc):
                csz = min(128, C - cc * 128)
                ft = pool.tile([128, TBIG], F32, tag="ft")
                nc.sync.dma_start(
                    out=ft[:csz],
                    in_=feat_d[cc * 128:cc * 128 + csz, ih * TBIG:(ih + 1) * TBIG],
                )
                junk = pool.tile([128, TBIG], F32, tag="junk")
                nc.vector.tensor_tensor_reduce(
                    out=junk[:csz],
                    in0=ft[:csz],
                    in1=wrep[:csz],
                    scale=1.0,
                    scalar=src[:csz, cc:cc + 1],
                    op0=mybir.AluOpType.mult,
                    op1=mybir.AluOpType.add,
                    accum_out=dst[:csz, cc:cc + 1],
                )
        fin = acc[nhw % 2]
        for cc in range(ncc):
            csz = min(128, C - cc * 128)
            nc.sync.dma_start(
                out=ctx_d[cc * 128:cc * 128 + csz][:, None], in_=fin[:csz, cc:cc + 1]
            )
    nc.compile()
    return nc


_CACHE = {}


def _get_nc(name, builder):
    if name not in _CACHE:
        _CACHE[name] = builder()
    return _CACHE[name]


def _host_prep(cnn_features_trans, hidden, alpha_sum, image_mask,
               W_h, b_h, conv_w, W_att, W_alpha):
    f32 = np.float32
    query = (hidden.astype(f32) @ W_h.astype(f32) + b_h.astype(f32))  # [8, 512]
    Mf0 = np.einsum("cij,ca->ija", conv_w[:, 0].astype(f32),
                    W_att.astype(f32)).reshape(TAPS, A)
    Mf = np.zeros((TAPSP, A), np.float32)
    Mf[:TAPS] = Mf0
    apad = np.pad(alpha_sum[:, 0].astype(f32),
                  ((0, 0), (KS // 2, KS // 2), (KS // 2, KS // 2)))
    from numpy.lib.stride_tricks import sliding_window_view
    sw = sliding_window_view(apad, (KS, KS), axis=(1, 2))  # [B, H, W, 11, 11]
    import ml_dtypes
    X8 = np.zeros((B, TAPSP, HW), ml_dtypes.bfloat16)
    X8[:, :TAPS] = sw.transpose(0, 3, 4, 1, 2).reshape(B, TAPS, HW)
    qp = np.ascontiguousarray(query.reshape(B, NCH, 128).transpose(0, 2, 1))
    Mf = Mf.astype(ml_dtypes.bfloat16)
    wa = np.zeros((128, NCH * 32), np.float32)
    wa[:, 0::32] = W_alpha[:, 0].astype(f32).reshape(NCH, 128).T
    wa = wa.astype(ml_dtypes.bfloat16)
    id128 = np.eye(128, dtype=ml_dtypes.bfloat16)
    tr = np.ascontiguousarray(cnn_features_trans.astype(f32).reshape(B, A, HW)).astype(ml_dtypes.bfloat16)
    am = np.ascontiguousarray(alpha_sum.astype(f32).reshape(B, HW))
    im = np.ascontiguousarray(image_mask.astype(f32).reshape(B, HW))
    return X8, tr, qp, Mf, wa, am, im, id128


def kernel(cnn_features, cnn_features_trans, hidden, alpha_sum, image_mask,
           W_h, b_h, conv_w, W_att, W_alpha, b_alpha):
    cnn_features = np.asarray(cnn_features)
    args = [np.asarray(a) for a in (cnn_features_trans, hidden, alpha_sum,
                                    image_mask, W_h, b_h, conv_w, W_att,
                                    W_alpha)]
    X8, tr, qp, Mf, wa, am, im, id128 = _host_prep(*args)

    nc = _get_nc("main", _build_main_nc)
    in_maps = [
        {"xt": X8[b], "tr": tr[b], "qp": qp[b], "mm": Mf, "wa": wa,
         "am": am[b], "im": im[b], "id128": id128}
        for b in range(B)
    ]
    res = bass_utils.run_bass_kernel_spmd(nc, in_maps, core_ids=list(range(B)))
    alpha = np.stack([res.results[b]["alpha_o"] for b in range(B)]).reshape(B, H, W)
    nas = np.stack([res.results[b]["nas_o"] for b in range(B)]).reshape(B, 1, H, W)

    # context_vector: alpha > 0.02 can hold at <= 49 positions (sum(alpha) <= 1);
    # for generic inputs it never holds and the exact result is 0.
    mask = alpha > np.float32(0.02)
    if mask.any():
        wv = (alpha * mask).reshape(B, HW).astype(np.float32)
        feat = np.ascontiguousarray(cnn_features.astype(np.float32).reshape(B, C, HW))
        ncc = _get_nc("ctx", _build_ctx_nc)
        in_maps2 = [{"feat": feat[b], "wv": wv[b]} for b in range(B)]
        res2 = bass_utils.run_bass_kernel_spmd(ncc, in_maps2, core_ids=list(range(B)))
        ctxv = np.stack([res2.results[b]["ctx_o"] for b in range(B)])
    else:
        ctxv = np.zeros((B, C), np.float32)

    return ctxv, alpha.astype(np.float32), nas.astype(np.float32)


# revision 21
# speedup vs baseline: 1.1517x; 1.0473x over previous
"""Trainium2 Bass kernel for the coverage-attention module.

Strategy (data-parallel over batch B=8, one batch per NeuronCore):
  - Host precompute (cheap, layout-only / tiny GEMMs):
      * query = hidden @ W_h + b_h                      [8, 512]
      * M     = einsum(conv_w, W_att) -> [121, 512]     (fuses the 11x11 conv
        with the channel-mixing einsum: coverage_alpha = im2col(alpha_sum) @ M)
      * X     = im2col of padded alpha_sum -> [8, 121, 16384]
  - Device (per core): conv-as-matmul + trans add + tanh + energy matvec on
    the tensor engine, then a local softmax (the global max subtraction is
    numerically equivalent to a per-sample max shift: softmax is shift
    invariant and the 1e-10 denominator guard perturbs at ~1e-13 relative,
    far below fp32 resolution).
  - context_vector = einsum(alpha * (alpha > 0.02), cnn_features).  Since
    sum(alpha) <= 1, at most 49 positions can exceed 0.02; for generic
    inputs none do and the result is exactly 0.  The mask is checked on the
    host from the returned alpha; only if any position passes is a second
    (dense matvec) kernel built and run.
"""

import numpy as np
from contextlib import ExitStack

import concourse.bass as bass
import concourse.tile as tile
from concourse import bacc, mybir, bass_isa
from concourse import bass_utils

B, C, H, W = 8, 684, 64, 256
HID, A, CONV_CH, KS = 256, 512, 512, 11
HW = H * W              # 16384
TAPS = KS * KS          # 121
TAPSP = 128             # taps padded to 128 for FWL (K=128)
NCH = A // 128          # 4 chunks of the attention dim
NT4 = 8                 # big DMA tiles along hw (2048 wide)
NTT = 4                 # compute subtiles per big tile (512 wide)
TBIG = HW // NT4        # 2048
TSUB = TBIG // NTT      # 512
F32 = mybir.dt.float32
BF16 = mybir.dt.bfloat16
AF = mybir.ActivationFunctionType
USE_PE_FRAC = False


def _build_main_nc():
    nc = bacc.Bacc("TRN2", target_bir_lowering=False, debug=False, num_devices=B)

    xt_d = nc.dram_tensor("xt", [TAPSP, HW], BF16, kind="ExternalInput").ap()
    tr_d = nc.dram_tensor("tr", [A, HW], BF16, kind="ExternalInput").ap()
    qp_d = nc.dram_tensor("qp", [128, NCH], F32, kind="ExternalInput").ap()
    mm_d = nc.dram_tensor("mm", [TAPSP, A], BF16, kind="ExternalInput").ap()
    wa_d = nc.dram_tensor("wa", [128, NCH * 32], BF16, kind="ExternalInput").ap()
    id_d = nc.dram_tensor("id128", [128, 128], BF16, kind="ExternalInput").ap()
    am_d = nc.dram_tensor("am", [HW], F32, kind="ExternalInput").ap()
    im_d = nc.dram_tensor("im", [HW], F32, kind="ExternalInput").ap()
    alpha_d = nc.dram_tensor("alpha_o", [HW], F32, kind="ExternalOutput").ap()
    nas_d = nc.dram_tensor("nas_o", [HW], F32, kind="ExternalOutput").ap()

    with tile.TileContext(nc) as tc, ExitStack() as ctx:
        consts = ctx.enter_context(tc.tile_pool(name="consts", bufs=1))
        xpool = ctx.enter_context(tc.tile_pool(name="xpool", bufs=3))
        trpool = ctx.enter_context(tc.tile_pool(name="trpool", bufs=16))
        prepool = ctx.enter_context(tc.tile_pool(name="prepool", bufs=3))
        scpool = ctx.enter_context(tc.tile_pool(name="scpool", bufs=6))
        evpool = ctx.enter_context(tc.tile_pool(name="evpool", bufs=2))
        smpool = ctx.enter_context(tc.tile_pool(name="smpool", bufs=1))
        cvps = ctx.enter_context(tc.tile_pool(name="cvps", bufs=3, space="PSUM"))
        epps = ctx.enter_context(tc.tile_pool(name="epps", bufs=2, space="PSUM"))

        mm_sb = consts.tile([TAPSP, A], BF16)
        nc.sync.dma_start(out=mm_sb, in_=mm_d)
        qp_sb = consts.tile([128, NCH], F32)
        nc.sync.dma_start(out=qp_sb, in_=qp_d)
        wa_sb = consts.tile([128, NCH * 32], BF16)
        nc.sync.dma_start(out=wa_sb, in_=wa_d)
        id_sb = consts.tile([128, 128], BF16)
        nc.sync.dma_start(out=id_sb, in_=id_d)
        im128 = consts.tile([128, 128], F32)
        nc.sync.dma_start(out=im128, in_=im_d.rearrange("(p f) -> p f", p=128))
        am128 = consts.tile([128, 128], F32)
        nc.sync.dma_start(out=am128, in_=am_d.rearrange("(p f) -> p f", p=128))

        ch_ts = []
        for c in range(NCH):
            ch_t = smpool.tile([128, 128], F32, name=f"ch{c}", tag=f"ch{c}")
            ch_ts.append(ch_t)

        for t4 in range(NT4):
            xt_t = xpool.tile([TAPSP, TBIG], BF16, tag="xt")
            tr_ts = [trpool.tile([128, TBIG], BF16, name=f"tr_t{c}", tag="tr")
                     for c in range(NCH)]
            if t4 == 0:
                # interleave X/trans halves so compute can ramp up sooner
                hw0, hw1 = 0, TBIG // 2
                nc.sync.dma_start(out=xt_t[:, :hw1], in_=xt_d[:, :hw1])
                nc.sync.dma_start(out=tr_ts[0][:, :hw1], in_=tr_d[0:128, :hw1])
                nc.sync.dma_start(out=xt_t[:, hw1:], in_=xt_d[:, hw1:TBIG])
                nc.sync.dma_start(out=tr_ts[0][:, hw1:], in_=tr_d[0:128, hw1:TBIG])
                for c in range(1, NCH):
                    nc.sync.dma_start(
                        out=tr_ts[c], in_=tr_d[c * 128:(c + 1) * 128, 0:TBIG])
            else:
                nc.sync.dma_start(out=xt_t,
                                  in_=xt_d[:, t4 * TBIG:(t4 + 1) * TBIG])
                for c in range(NCH):
                    nc.sync.dma_start(
                        out=tr_ts[c],
                        in_=tr_d[c * 128:(c + 1) * 128,
                                 t4 * TBIG:(t4 + 1) * TBIG],
                    )
            # sc_ap(c, tt) -> AP of the [128, 512] tanh(score) slice
            sc_aps = {}
            for c in range(NCH):
                use_pe = USE_PE_FRAC and (t4 * NCH + c) % 2 == 0
                if use_pe:
                    # trans-add on the tensor engine (identity accumulate),
                    # tanh straight out of PSUM
                    for j in range(TBIG // 1024):
                        lo = j * 1024
                        cv = cvps.tile([128, 1024], F32, tag="cv")
                        for s in range(2):
                            nc.tensor.matmul(
                                cv[:, s * TSUB:(s + 1) * TSUB],
                                mm_sb[:, c * 128:(c + 1) * 128],
                                xt_t[:, lo + s * TSUB:lo + (s + 1) * TSUB],
                                start=True,
                                stop=True,
                            )
                        for s in range(2):
                            nc.tensor.matmul(
                                cv[:, s * TSUB:(s + 1) * TSUB],
                                id_sb,
                                tr_ts[c][:, lo + s * TSUB:lo + (s + 1) * TSUB],
                                start=False,
                                stop=True,
                                skip_group_check=True,
                            )
                        sc1k = scpool.tile([128, 1024], BF16, tag="sc1k")
                        nc.scalar.activation(
                            sc1k, cv, AF.Tanh, bias=qp_sb[:, c:c + 1], scale=1.0
                        )
                        sc_aps[(c, 2 * j)] = sc1k[:, 0:TSUB]
                        sc_aps[(c, 2 * j + 1)] = sc1k[:, TSUB:1024]
                else:
                    # trans-add on the vector engine, tanh from SBUF
                    pre2k = prepool.tile([128, TBIG], F32, tag="pre")
                    for j in range(TBIG // 1024):
                        lo = j * 1024
                        cv = cvps.tile([128, 1024], F32, tag="cv")
                        for s in range(2):
                            nc.tensor.matmul(
                                cv[:, s * TSUB:(s + 1) * TSUB],
                                mm_sb[:, c * 128:(c + 1) * 128],
                                xt_t[:, lo + s * TSUB:lo + (s + 1) * TSUB],
                                start=True,
                                stop=True,
                            )
                        nc.vector.tensor_add(
                            pre2k[:, lo:lo + 1024], cv, tr_ts[c][:, lo:lo + 1024]
                        )
                    sc2k = scpool.tile([128, TBIG], BF16, tag="sc")
                    nc.scalar.activation(
                        sc2k, pre2k, AF.Tanh, bias=qp_sb[:, c:c + 1], scale=1.0
                    )
                    for tt in range(NTT):
                        sc_aps[(c, tt)] = sc2k[:, tt * TSUB:(tt + 1) * TSUB]
            evs = evpool.tile([97, TBIG], F32, tag="evs")
            for tt in range(NTT):
                ep = epps.tile([128, TSUB], F32, tag="ep")
                for c in range(NCH):
                    nc.tensor.matmul(
                        ep[32 * c:32 * (c + 1), :],
                        wa_sb[:, c * 32:(c + 1) * 32],
                        sc_aps[(c, tt)],
                        start=True,
                        stop=True,
                        tile_position=(0, 32 * c),
                    )
                nc.scalar.copy(evs[:, tt * TSUB:(tt + 1) * TSUB], ep[0:97, :])
            for c in range(NCH):
                nc.sync.dma_start(
                    out=ch_ts[c][16 * t4:16 * (t4 + 1), :],
                    in_=evs[32 * c:32 * c + 1, :])

        # --- softmax over the 16384 positions, done on a [128, 128] layout ---
        e01 = smpool.tile([128, 128], F32)
        nc.vector.tensor_add(e01, ch_ts[0], ch_ts[1])
        e23 = smpool.tile([128, 128], F32)
        nc.vector.tensor_add(e23, ch_ts[2], ch_ts[3])
        e128 = smpool.tile([128, 128], F32)
        nc.vector.tensor_add(e128, e01, e23)
        mx = smpool.tile([128, 1], F32)
        nc.vector.reduce_max(mx, e128, axis=mybir.AxisListType.X)
        mxa = smpool.tile([128, 1], F32)
        nc.gpsimd.partition_all_reduce(
            mxa, mx, channels=128, reduce_op=bass_isa.ReduceOp.max
        )
        nmx = smpool.tile([128, 1], F32)
        nc.vector.tensor_scalar_mul(nmx, mxa, -1.0)
        ex = smpool.tile([128, 128], F32)
        nc.scalar.activation(ex, e128, AF.Exp, bias=nmx, scale=1.0)
        ee = smpool.tile([128, 128], F32)
        nc.vector.tensor_mul(ee, ex, im128)
        sm = smpool.tile([128, 1], F32)
        nc.vector.reduce_sum(sm, ee, axis=mybir.AxisListType.X)
        sma = smpool.tile([128, 1], F32)
        nc.gpsimd.partition_all_reduce(
            sma, sm, channels=128, reduce_op=bass_isa.ReduceOp.add
        )
        smb = smpool.tile([128, 1], F32)
        nc.vector.tensor_scalar_add(smb, sma, 1e-10)
        rr = smpool.tile([128, 1], F32)
        nc.vector.reciprocal(rr, smb)
        al = smpool.tile([128, 128], F32)
        nc.vector.tensor_scalar_mul(al, ee, rr)
        nas = smpool.tile([128, 128], F32)
        nc.vector.tensor_add(nas, al, am128)
        nc.sync.dma_start(out=alpha_d.rearrange("(p f) -> p f", p=128), in_=al)
        nc.sync.dma_start(out=nas_d.rearrange("(p f) -> p f", p=128), in_=nas)

    nc.compile()
    return nc


def _build_ctx_nc():
    """Fallback dense context matvec: ctx[c] = sum_hw wv[hw] * feat[c, hw]."""
    nc = bacc.Bacc("TRN2", target_bir_lowering=False, debug=False, num_devices=B)
    feat_d = nc.dram_tensor("feat", [C, HW], F32, kind="ExternalInput").ap()
    wv_d = nc.dram_tensor("wv", [HW], F32, kind="ExternalInput").ap()
    ctx_d = nc.dram_tensor("ctx_o", [C], F32, kind="ExternalOutput").ap()

    ncc = (C + 127) // 128  # 6 chunks (5 x 128 + 44)
    nhw = 8                 # hw chunks of 2048

    with tile.TileContext(nc) as tc, ExitStack() as ctx:
        pool = ctx.enter_context(tc.tile_pool(name="p", bufs=3))
        accp = ctx.enter_context(tc.tile_pool(name="acc", bufs=1))
        acc = [accp.tile([128, ncc], F32, name=f"acc{i}", tag=f"acc{i}")
               for i in range(2)]
        nc.vector.memset(acc[0], 0.0)
        nc.vector.memset(acc[1], 0.0)
        for ih in range(nhw):
            wv_sb = pool.tile([1, TBIG], F32, tag="wv")
            nc.sync.dma_start(out=wv_sb, in_=wv_d[ih * TBIG:(ih + 1) * TBIG][None, :])
            wrep = pool.tile([128, TBIG], F32, tag="wrep")
            nc.gpsimd.partition_broadcast(wrep, wv_sb, channels=128)
            src, dst = acc[ih % 2], acc[(ih + 1) % 2]
            for cc in range(ncc):
                csz = min(128, C - cc * 128)
                ft = pool.tile([128, TBIG], F32, tag="ft")
                nc.sync.dma_start(
                    out=ft[:csz],
                    in_=feat_d[cc * 128:cc * 128 + csz, ih * TBIG:(ih + 1) * TBIG],
                )
                junk = pool.tile([128, TBIG], F32, tag="junk")
                nc.vector.tensor_tensor_reduce(
                    out=junk[:csz],
                    in0=ft[:csz],
                    in1=wrep[:csz],
                    scale=1.0,
                    scalar=src[:csz, cc:cc + 1],
                    op0=mybir.AluOpType.mult,
                    op1=mybir.AluOpType.add,
                    accum_out=dst[:csz, cc:cc + 1],
                )
        fin = acc[nhw % 2]
        for cc in range(ncc):
            csz = min(128, C - cc * 128)
            nc.sync.dma_start(
                out=ctx_d[cc * 128:cc * 128 + csz][:, None], in_=fin[:csz, cc:cc + 1]
            )
    nc.compile()
    return nc


_CACHE = {}


def _get_nc(name, builder):
    if name not in _CACHE:
        _CACHE[name] = builder()
    return _CACHE[name]


def _host_prep(cnn_features_trans, hidden, alpha_sum, image_mask,
               W_h, b_h, conv_w, W_att, W_alpha):
    f32 = np.float32
    query = (hidden.astype(f32) @ W_h.astype(f32) + b_h.astype(f32))  # [8, 512]
    Mf0 = np.einsum("cij,ca->ija", conv_w[:, 0].astype(f32),
                    W_att.astype(f32)).reshape(TAPS, A)
    Mf = np.zeros((TAPSP, A), np.float32)
    Mf[:TAPS] = Mf0
    apad = np.pad(alpha_sum[:, 0].astype(f32),
                  ((0, 0), (KS // 2, KS // 2), (KS // 2, KS // 2)))
    from numpy.lib.stride_tricks import sliding_window_view
    sw = sliding_window_view(apad, (KS, KS), axis=(1, 2))  # [B, H, W, 11, 11]
    import ml_dtypes
    X8 = np.zeros((B, TAPSP, HW), ml_dtypes.bfloat16)
    X8[:, :TAPS] = sw.transpose(0, 3, 4, 1, 2).reshape(B, TAPS, HW)
    qp = np.ascontiguousarray(query.reshape(B, NCH, 128).transpose(0, 2, 1))
    Mf = Mf.astype(ml_dtypes.bfloat16)
    wa = np.zeros((128, NCH * 32), np.float32)
    wa[:, 0::32] = W_alpha[:, 0].astype(f32).reshape(NCH, 128).T
    wa = wa.astype(ml_dtypes.bfloat16)
    id128 = np.eye(128, dtype=ml_dtypes.bfloat16)
    tr = np.ascontiguousarray(cnn_features_trans.astype(f32).reshape(B, A, HW)).astype(ml_dtypes.bfloat16)
    am = np.ascontiguousarray(alpha_sum.astype(f32).reshape(B, HW))
    im = np.ascontiguousarray(image_mask.astype(f32).reshape(B, HW))
    return X8, tr, qp, Mf, wa, am, im, id128


def kernel(cnn_features, cnn_features_trans, hidden, alpha_sum, image_mask,
           W_h, b_h, conv_w, W_att, W_alpha, b_alpha):
    cnn_features = np.asarray(cnn_features)
    args = [np.asarray(a) for a in (cnn_features_trans, hidden, alpha_sum,
                                    image_mask, W_h, b_h, conv_w, W_att,
                                    W_alpha)]
    X8, tr, qp, Mf, wa, am, im, id128 = _host_prep(*args)

    nc = _get_nc("main", _build_main_nc)
    in_maps = [
        {"xt": X8[b], "tr": tr[b], "qp": qp[b], "mm": Mf, "wa": wa,
         "am": am[b], "im": im[b], "id128": id128}
        for b in range(B)
    ]
    res = bass_utils.run_bass_kernel_spmd(nc, in_maps, core_ids=list(range(B)))
    alpha = np.stack([res.results[b]["alpha_o"] for b in range(B)]).reshape(B, H, W)
    nas = np.stack([res.results[b]["nas_o"] for b in range(B)]).reshape(B, 1, H, W)

    # context_vector: alpha > 0.02 can hold at <= 49 positions (sum(alpha) <= 1);
    # for generic inputs it never holds and the exact result is 0.
    mask = alpha > np.float32(0.02)
    if mask.any():
        wv = (alpha * mask).reshape(B, HW).astype(np.float32)
        feat = np.ascontiguousarray(cnn_features.astype(np.float32).reshape(B, C, HW))
        ncc = _get_nc("ctx", _build_ctx_nc)
        in_maps2 = [{"feat": feat[b], "wv": wv[b]} for b in range(B)]
        res2 = bass_utils.run_bass_kernel_spmd(ncc, in_maps2, core_ids=list(range(B)))
        ctxv = np.stack([res2.results[b]["ctx_o"] for b in range(B)])
    else:
        ctxv = np.zeros((B, C), np.float32)

    return ctxv, alpha.astype(np.float32), nas.astype(np.float32)
